# revision 1
# baseline (speedup 1.0000x reference)
"""Trainium2 Bass kernel for DecoupledMVRowSelfAttnProcessor.

Sharding: tensor-parallel over heads — 8 cores x 2 heads each.
Each core computes, for its 128-wide head slice:
  - branch1 (self-attn) + branch3 (ref cross-attn) partial output -> out_main
  - multi-view row-attention partial output -> out_mv
Host sums the 16 partials + residual + biases.

Dtypes: float32r (tf32-like multiply, full PE rate at N>=256) for the main
branches; bf16 for the mv branch matmuls.
"""
import sys

for _p in ('/opt/trn_rl_repo',):
    if _p not in sys.path:
        sys.path.insert(0, _p)

import numpy as np
import ml_dtypes

# ---- problem constants (hardcoded per contest rules) ----
B, S, C = 12, 1024, 1024
HEADS, D = 16, 64
NV, IH, IW = 6, 32, 32
T = B * S                # 12288 tokens
HL = 2                   # heads per core
D2 = HL * D              # 128: per-core head-slice width
N_CORES = 8
CT = 8                   # C tiles of 128
TCHUNK = 512             # token chunk for projections
NCH = S // TCHUNK        # 2 chunks per batch

_BUILT = None
TRACE = False
LAST_RESULTS = None


def _build():
    import concourse.bass as bass  # noqa: F401
    from concourse import bacc
    import concourse.mybir as mybir
    from concourse.tile import TileContext

    f32 = mybir.dt.float32
    f32r = mybir.dt.float32r
    bf16 = mybir.dt.bfloat16
    EXP = mybir.ActivationFunctionType.Exp
    MULT = mybir.AluOpType.mult
    P = 128

    nc = bacc.Bacc("TRN2", target_bir_lowering=False, debug=False)

    # ---- DRAM tensors ----
    hsT = nc.dram_tensor("hsT", [C, T], f32r, kind="ExternalInput")
    refT = nc.dram_tensor("refT", [C, T], f32r, kind="ExternalInput")
    w_tp = {}
    for name in ("wq", "wk", "wqm", "wkm", "wqr", "wkr", "wvr"):
        w_tp[name] = nc.dram_tensor(name, [P, CT, D2], f32r, kind="ExternalInput")
    wv_cat = nc.dram_tensor("wv_cat", [P, CT, 2 * D2], f32r, kind="ExternalInput")
    wout = nc.dram_tensor("wout", [D2, C], f32r, kind="ExternalInput")
    woutr = nc.dram_tensor("woutr", [D2, C], f32r, kind="ExternalInput")
    woutm = nc.dram_tensor("woutm", [D2, C], bf16, kind="ExternalInput")
    ident = nc.dram_tensor("ident", [P, P], f32r, kind="ExternalInput")
    out_main = nc.dram_tensor("out_main", [T, C], f32r, kind="ExternalOutput")
    out_mv = nc.dram_tensor("out_mv", [T, C], f32r, kind="ExternalOutput")

    hsT_r = hsT.rearrange("(ct p) t -> p ct t", p=P)
    refT_r = refT.rearrange("(ct p) t -> p ct t", p=P)

    with TileContext(nc) as tc:
        with tc.tile_pool(name="wpool", bufs=1) as wpool, \
             tc.tile_pool(name="const", bufs=1) as cpool, \
             tc.tile_pool(name="hsref", bufs=2) as hpool, \
             tc.tile_pool(name="projs", bufs=2) as ppool, \
             tc.tile_pool(name="projs1", bufs=1) as ppool1, \
             tc.tile_pool(name="psP", bufs=2, space="PSUM") as psP, \
             tc.tile_pool(name="vprime", bufs=1) as vpool, \
             tc.tile_pool(name="apool", bufs=2) as apool, \
             tc.tile_pool(name="attn", bufs=1) as atpool, \
             tc.tile_pool(name="mv", bufs=1) as mvpool, \
             tc.tile_pool(name="mvsm", bufs=2) as mvsm, \
             tc.tile_pool(name="mvs1", bufs=1) as mvs1, \
             tc.tile_pool(name="outst", bufs=2) as opool, \
             tc.tile_pool(name="psS", bufs=2, space="PSUM") as psS, \
             tc.tile_pool(name="psB", bufs=2, space="PSUM") as psB:

            lp = nc.allow_low_precision(
                reason="f32r/bf16 tiles carry fp32-accumulated values")
            lp.__enter__()

            # ---- resident weights ----
            wt = {k: wpool.tile([P, CT, D2], f32r, tag=k, name=k) for k in w_tp}
            for k, dram in w_tp.items():
                nc.sync.dma_start(wt[k][:], dram[:])
            t_wv = wpool.tile([P, CT, 2 * D2], f32r, tag="wv_cat")
            nc.sync.dma_start(t_wv[:], wv_cat[:])
            t_wout = wpool.tile([D2, C], f32r, tag="wout")
            t_woutr = wpool.tile([D2, C], f32r, tag="woutr")
            t_woutm = wpool.tile([D2, C], bf16, tag="woutm")
            nc.sync.dma_start(t_wout[:], wout[:])
            nc.sync.dma_start(t_woutr[:], woutr[:])
            nc.sync.dma_start(t_woutm[:], woutm[:])
            t_id = wpool.tile([P, P], f32r, tag="ident")
            nc.sync.dma_start(t_id[:], ident[:])

            # ---- constants ----
            # f32r/bf16 matmul operands must come from same-dtype writers
            # (BIR "rounded to FP32r" rule) -> init via DVE copies from f32
            # zero/one scratch tiles, never via bitcast memsets.
            zf32 = cpool.tile([P, P], f32, tag="zf32")
            nc.any.memset(zf32[:], 0.0)
            of32 = cpool.tile([P, 64], f32, tag="of32")
            nc.any.memset(of32[:], 1.0)
            sel65 = cpool.tile([65, P], f32r, tag="sel65")
            nc.vector.tensor_copy(sel65[:], zf32[0:65, 0:P])
            nc.vector.tensor_copy(sel65[0:1, 0:64], of32[0:1, 0:64])
            nc.vector.tensor_copy(sel65[64:65, 64:128], of32[64:65, 0:64])
            ones128 = cpool.tile([1, P], bf16, tag="ones128")
            nc.vector.tensor_copy(ones128[:, 0:64], of32[0:1, :])
            nc.vector.tensor_copy(ones128[:, 64:128], of32[0:1, :])
            den_b1 = cpool.tile([65, S], f32r, tag="den_b1")
            den_rf = cpool.tile([65, S], f32r, tag="den_rf")
            for dd in (den_b1, den_rf):
                for qq in range(8):
                    nc.vector.tensor_copy(dd[:, qq * P:(qq + 1) * P],
                                          zf32[0:65, :])

            # ---- persistent per-branch tensors ----
            vps = {}
            for nm in ("v1A", "v1B", "vrA", "vrB"):
                vps[nm] = vpool.tile([P, CT, 65], f32r, tag=nm, name=nm)
                nc.vector.tensor_copy(
                    vps[nm][:, :, 64:65].rearrange("p t o -> p (t o)"),
                    of32[:, 0:CT])

            # [d2, row, view, col]: bank slice [hd, r, :, :] is contiguous (192)
            qmg = mvpool.tile([P, IH, NV, IW], bf16, tag="qmg")
            kmg = mvpool.tile([P, IH, NV, IW], bf16, tag="kmg")
            bank_lo = mvpool.tile([P, HL, IH, 65], bf16, tag="bank_lo")
            bank_hi = mvpool.tile([64, HL, IH, 65], bf16, tag="bank_hi")
            nc.vector.tensor_copy(
                bank_lo[:, :, :, 64:65].rearrange("p a b o -> p (a b o)"),
                of32[:, 0:HL * IH])
            nc.vector.tensor_copy(
                bank_hi[:, :, :, 64:65].rearrange("p a b o -> p (a b o)"),
                of32[0:64, 0:HL * IH])
            attn_mv_g = mvpool.tile([P, NV, CT, P], bf16, tag="attn_mv_g")

            def transposed_proj(dst, wtile, src_tile, ch):
                ps = psP.tile([P, TCHUNK], f32, tag="psP")
                for kt in range(CT):
                    nc.tensor.matmul(ps[:], wtile[:, kt, :], src_tile[:, kt, :],
                                     start=(kt == 0), stop=(kt == CT - 1))
                nc.vector.tensor_copy(dst[:, ch * TCHUNK:(ch + 1) * TCHUNK], ps[:])

            def attention(q_T, k_T, vpA, vpB, attn_cat, den65):
                for j in (0, 1):
                    hd = slice(64 * j, 64 * j + 64)
                    vp = vpA if j == 0 else vpB
                    po0 = psB.tile([65, 512], f32, tag="psB")
                    po1 = psB.tile([65, 512], f32, tag="psB")
                    for kt in range(CT):
                        ps_s = psS.tile([P, S], f32, tag="psS")
                        nc.tensor.matmul(ps_s[:, 0:512], k_T[hd, kt * P:(kt + 1) * P],
                                         q_T[hd, 0:512], start=True, stop=True)
                        nc.tensor.matmul(ps_s[:, 512:1024], k_T[hd, kt * P:(kt + 1) * P],
                                         q_T[hd, 512:1024], start=True, stop=True)
                        a_kt = apool.tile([P, S], f32r, tag="akt")
                        nc.scalar.activation(a_kt[:], ps_s[:], EXP, scale=0.125)
                        nc.tensor.matmul(po0[:], vp[:, kt, :], a_kt[:, 0:512],
                                         start=(kt == 0), stop=(kt == CT - 1))
                        nc.tensor.matmul(po1[:], vp[:, kt, :], a_kt[:, 512:1024],
                                         start=(kt == 0), stop=(kt == CT - 1))
                    nc.vector.tensor_copy(attn_cat[hd, 0:512], po0[0:64, :])
                    nc.vector.tensor_copy(attn_cat[hd, 512:1024], po1[0:64, :])
                    nc.vector.tensor_copy(den65[64 * j:64 * j + 1, 0:512], po0[64:65, :])
                    nc.vector.tensor_copy(den65[64 * j:64 * j + 1, 512:1024],
                                          po1[64:65, :])
                ps_inv = psS.tile([P, S], f32, tag="psS")  # bcast dens
                for ch2 in (0, 1):
                    nc.tensor.matmul(ps_inv[:, ch2 * 512:(ch2 + 1) * 512], sel65[:],
                                     den65[:, ch2 * 512:(ch2 + 1) * 512],
                                     start=True, stop=True)
                inv = atpool.tile([P, S], f32, tag="inv")
                nc.vector.reciprocal_approx_fast(out=inv[:], in_=ps_inv[:])
                nc.vector.tensor_mul(out=attn_cat[:], in0=attn_cat[:], in1=inv[:])

            # =================== main batch loop ===================
            for i in range(B):
                g, v = i // NV, i % NV
                tok0 = i * S

                q_T = ppool.tile([P, S], f32r, tag="qT")
                k_T = ppool.tile([P, S], f32r, tag="kT")
                qr_T = ppool1.tile([P, S], f32r, tag="qT2")
                kr_T = ppool1.tile([P, S], f32r, tag="kT2")
                vr_T = ppool1.tile([P, S], f32r, tag="vrT")
                vm_nat = mvs1.tile([P, CT, P], bf16, tag="vm_nat")

                # ---- P1a: hs-based projections ----
                for ch in range(NCH):
                    hs_t = hpool.tile([P, CT, TCHUNK], f32r, tag="hsref")
                    nc.sync.dma_start(
                        hs_t[:],
                        hsT_r[:, :, tok0 + ch * TCHUNK: tok0 + (ch + 1) * TCHUNK])
                    transposed_proj(q_T, wt["wq"], hs_t, ch)
                    transposed_proj(k_T, wt["wk"], hs_t, ch)
                    transposed_proj(qr_T, wt["wqr"], hs_t, ch)
                    for wname, dstg in (("wqm", qmg), ("wkm", kmg)):
                        ps = psP.tile([P, TCHUNK], f32, tag="psP")
                        for kt in range(CT):
                            nc.tensor.matmul(ps[:], wt[wname][:, kt, :],
                                             hs_t[:, kt, :],
                                             start=(kt == 0), stop=(kt == CT - 1))
                        nrow = TCHUNK // IW
                        nc.vector.tensor_copy(
                            dstg[:, ch * nrow:(ch + 1) * nrow, v, :],
                            ps[:].rearrange("p (r c) -> p r c", c=IW))
                    # natural-orientation V (v1 f32r + vm bf16), N=256
                    for tt in range(ch * 4, ch * 4 + 4):
                        psv = psP.tile([P, 2 * D2], f32, tag="psP")
                        off = tt * P - ch * TCHUNK
                        for kt in range(CT):
                            nc.tensor.matmul(psv[:], hs_t[:, kt, off:off + P],
                                             t_wv[:, kt, :],
                                             start=(kt == 0), stop=(kt == CT - 1))
                        nc.vector.tensor_copy(vps["v1A"][:, tt, 0:64], psv[:, 0:64])
                        nc.vector.tensor_copy(vps["v1B"][:, tt, 0:64], psv[:, 64:128])
                        nc.vector.tensor_copy(vm_nat[:, tt, :], psv[:, 128:256])

                # vm_nat -> row banks (strided SBUF->SBUF DMAs)
                bnk = bank_lo if v < 4 else bank_hi
                vv = v if v < 4 else v - 4
                for rl in range(4):
                    for hdI in range(HL):
                        nc.sync.dma_start(
                            bnk[32 * vv:32 * vv + 32, hdI, rl::4, 0:64],
                            vm_nat[rl * 32:(rl + 1) * 32, :,
                                   hdI * 64:(hdI + 1) * 64])

                # ---- P1b: ref-based projections ----
                for ch in range(NCH):
                    rf_t = hpool.tile([P, CT, TCHUNK], f32r, tag="hsref")
                    nc.sync.dma_start(
                        rf_t[:],
                        refT_r[:, :, tok0 + ch * TCHUNK: tok0 + (ch + 1) * TCHUNK])
                    transposed_proj(kr_T, wt["wkr"], rf_t, ch)
                    transposed_proj(vr_T, wt["wvr"], rf_t, ch)
                for tt in range(CT):
                    pst = psP.tile([P, P], f32r, tag="psP")
                    nc.tensor.transpose(pst[:], vr_T[:, tt * P:(tt + 1) * P], t_id[:])
                    nc.vector.tensor_copy(vps["vrA"][:, tt, 0:64], pst[:, 0:64])
                    nc.vector.tensor_copy(vps["vrB"][:, tt, 0:64], pst[:, 64:128])

                # ---- P2: branch 1 & 3 attention ----
                attn1 = atpool.tile([P, S], f32r, tag="attn1")
                attnr = atpool.tile([P, S], f32r, tag="attnr")
                attention(q_T, k_T, vps["v1A"], vps["v1B"], attn1, den_b1)
                attention(qr_T, kr_T, vps["vrA"], vps["vrB"], attnr, den_rf)

                # ---- out-projection (branches 1+3 accumulate) ----
                for tt in range(CT):
                    pso = psS.tile([P, C], f32, tag="psS")
                    for ch2 in (0, 1):
                        sl = slice(ch2 * 512, (ch2 + 1) * 512)
                        nc.tensor.matmul(pso[:, sl], attn1[:, tt * P:(tt + 1) * P],
                                         t_wout[:, sl], start=True, stop=False)
                        nc.tensor.matmul(pso[:, sl], attnr[:, tt * P:(tt + 1) * P],
                                         t_woutr[:, sl], start=False, stop=True)
                    for ch2 in (0, 1):
                        sl = slice(ch2 * 512, (ch2 + 1) * 512)
                        ost = opool.tile([P, 512], f32r, tag="ost")
                        nc.vector.tensor_copy(ost[:], pso[:, sl])
                        nc.sync.dma_start(
                            out_main[tok0 + tt * P: tok0 + (tt + 1) * P, sl], ost[:])

                # =============== mv attention at group end ===============
                if v == NV - 1:
                    for j in (0, 1):
                        hd = slice(64 * j, 64 * j + 64)
                        for pq in range(16):  # row pairs
                            ps_lo = psP.tile([P, 2, 192], f32, tag="psP")
                            ps_hi = psP.tile([64, 2, 192], f32, tag="psP")
                            for rl in range(2):
                                r = 2 * pq + rl
                                nc.tensor.matmul(ps_lo[:, rl, :],
                                                 kmg[hd, r, 0:4, :],
                                                 qmg[hd, r, :, :],
                                                 start=True, stop=True)
                                nc.tensor.matmul(ps_hi[:, rl, :],
                                                 kmg[hd, r, 4:6, :],
                                                 qmg[hd, r, :, :],
                                                 start=True, stop=True)
                            a_lo = mvs1.tile([P, 2, 192], bf16, tag="a_lo")
                            a_hi = mvs1.tile([64, 2, 192], bf16, tag="a_hi")
                            nc.scalar.activation(a_lo[:], ps_lo[:], EXP, scale=0.125)
                            nc.scalar.activation(a_hi[:], ps_hi[:], EXP, scale=0.125)
                            po = psB.tile([65, 2, 192], f32, tag="psB")
                            for rl in range(2):
                                r = 2 * pq + rl
                                nc.tensor.matmul(po[:, rl, :], bank_lo[:, j, r, :],
                                                 a_lo[:, rl, :],
                                                 start=True, stop=False)
                                nc.tensor.matmul(po[:, rl, :], bank_hi[:, j, r, :],
                                                 a_hi[:, rl, :],
                                                 start=False, stop=True)
                            qden = mvs1.tile([1, 384], f32, tag="qden")
                            nc.vector.tensor_copy(
                                qden[:], po[64:65, :, :].rearrange("p r c -> p (r c)"))
                            nc.vector.reciprocal_approx_fast(out=qden[:], in_=qden[:])
                            qinv16 = mvs1.tile([1, 384], bf16, tag="qinv16")
                            nc.vector.tensor_copy(qinv16[:], qden[:])
                            ps_binv = psP.tile([64, 384], f32, tag="psP")
                            nc.tensor.matmul(ps_binv[:], ones128[:, 0:64],
                                             qinv16[:], start=True, stop=True)
                            binv = mvsm.tile([64, 384], bf16, tag="binv")
                            nc.vector.tensor_copy(binv[:], ps_binv[:])
                            # pair of rows lands in tok-tile t=pq//2 at offset 0/64
                            t_t = pq // 2
                            off = (pq % 2) * 64
                            dst = attn_mv_g[hd, :, t_t, off:off + 64].rearrange(
                                "p v (r c) -> p v r c", r=2)
                            nc.vector.tensor_tensor(
                                dst,
                                po[0:64, :, :].rearrange("p r (v c) -> p v r c", v=NV),
                                binv[:].rearrange("p (r v c) -> p v r c", r=2, v=NV),
                                MULT)

                    # mv out-projection for the 6 batches of this group
                    for v2 in range(NV):
                        tok0b = (g * NV + v2) * S
                        for tt in range(CT):
                            psm = psS.tile([P, C], f32, tag="psS")
                            for ch2 in (0, 1):
                                sl = slice(ch2 * 512, (ch2 + 1) * 512)
                                nc.tensor.matmul(psm[:, sl],
                                                 attn_mv_g[:, v2, tt, :],
                                                 t_woutm[:, sl],
                                                 start=True, stop=True)
                            for ch2 in (0, 1):
                                sl = slice(ch2 * 512, (ch2 + 1) * 512)
                                ostm = opool.tile([P, 512], f32r, tag="ost")
                                nc.vector.tensor_copy(ostm[:], psm[:, sl])
                                nc.sync.dma_start(
                                    out_mv[tok0b + tt * P: tok0b + (tt + 1) * P, sl],
                                    ostm[:])

            lp.__exit__(None, None, None)

    nc.compile()
    return nc


def _get_built():
    global _BUILT
    if _BUILT is None:
        _BUILT = _build()
    return _BUILT


def kernel(**inputs):
    nc = _get_built()
    from concourse.bass_utils import run_bass_kernel_spmd

    hs = np.asarray(inputs["hidden_states"], np.float32)
    ref = np.asarray(inputs["ref_hidden_states"], np.float32)
    hsT = np.ascontiguousarray(hs.reshape(T, C).T)
    refT = np.ascontiguousarray(ref.reshape(T, C).T)
    ident = np.eye(128, dtype=np.float32)

    def tp_w(w, hc):  # [C, 128] slice -> [128 Cpart, 8 Ctile, 128]
        return np.ascontiguousarray(
            np.asarray(w, np.float32)[:, hc].reshape(CT, 128, D2).transpose(1, 0, 2))

    in_maps = []
    for c in range(N_CORES):
        hc = slice(D2 * c, D2 * (c + 1))
        wvc = np.concatenate(
            [np.asarray(inputs["Wv"], np.float32)[:, hc],
             np.asarray(inputs["Wv_mv"], np.float32)[:, hc]], axis=1)
        in_maps.append({
            "hsT": hsT, "refT": refT, "ident": ident,
            "wq": tp_w(inputs["Wq"], hc), "wk": tp_w(inputs["Wk"], hc),
            "wqm": tp_w(inputs["Wq_mv"], hc), "wkm": tp_w(inputs["Wk_mv"], hc),
            "wqr": tp_w(inputs["Wq_ref"], hc), "wkr": tp_w(inputs["Wk_ref"], hc),
            "wvr": tp_w(inputs["Wv_ref"], hc),
            "wv_cat": np.ascontiguousarray(
                wvc.reshape(CT, 128, 2 * D2).transpose(1, 0, 2)),
            "wout": np.ascontiguousarray(
                np.asarray(inputs["Wout"], np.float32)[hc, :]),
            "woutr": np.ascontiguousarray(
                np.asarray(inputs["Wout_ref"], np.float32)[hc, :]),
            "woutm": np.ascontiguousarray(
                np.asarray(inputs["Wout_mv"], np.float32)[hc, :]).astype(
                    ml_dtypes.bfloat16),
        })

    global LAST_RESULTS
    kwargs = {}
    if TRACE:
        kwargs = dict(trace=True, trace_cores=list(range(N_CORES)))
    res = run_bass_kernel_spmd(nc, in_maps, core_ids=list(range(N_CORES)), **kwargs)
    LAST_RESULTS = res

    acc = np.zeros((T, C), np.float32)
    for r in res.results:
        acc += r["out_main"]
        acc += r["out_mv"]
    acc += hs.reshape(T, C)
    acc += (np.asarray(inputs["bout"], np.float32)
            + np.asarray(inputs["bout_mv"], np.float32)
            + np.asarray(inputs["bout_ref"], np.float32))[None, :]
    return acc.reshape(B, S, C)

